# revision 65
# baseline (speedup 1.0000x reference)
"""Trainium2 Bass kernel for nn_Attention_30305289240928.

Single-layer causal attention with RMSNorm prologue:
    xn = x * rsqrt(mean(x^2) + eps)           (RMSNorm, no weight)
    qkv = xn @ wqkv.T  -> per-head q, k, v    (16 heads, head_dim 128)
    out = softmax(causal(q k^T / sqrt(128))) v, concat heads, @ wo.T

Sharding: head-parallel tensor parallel over 8 NeuronCores.
Core c owns heads 2c, 2c+1 (wqkv rows c*768:(c+1)*768) and the matching
wo input-columns c*256:(c+1)*256. Each core computes a full-shape partial
of the output projection; the host sums the 8 partials.

Device-side design (v3, fp8/fp16 mixed precision, fused phases):
  - QKV projection runs on fp8e4m3 DoubleRow matmuls (2 k-tiles per
    instruction at 0.5 cycles/col = 4x fp32r element throughput). Inputs
    are host-quantized into scaled hi/lo pairs (x*16, w*64, hi and lo at
    the same scale); the product uses the 3-term correction
    Wh@Xh + Wl@Xh + Wh@Xl (dropped Wl@Xl term ~1e-3 relative).
  - RMSNorm: squares of x-hi on ACT+DVE one block ahead; per-token sums
    via 1-column transposed-stationary matmuls (~1 PE cycle each). The
    broadcast s_bc (= s/32, folding the fp8 descale) is built with a
    tiny fp16 transpose plus 1-partition broadcast matmuls. Both Q and
    K evict with the s_bc multiply, so exp's scale is a constant and
    exp batches over multi-block score groups.
  - Attention is fp16 (fp32r-grade mantissa, full PE rate at any N,
    enabling exact-causal column trimming). Scores are computed
    transposed, S.T[kt, qt]; causal masking is folded into the score
    PSUM accumulation as a -290k rank-128 add (exp underflows to 0), so
    no vector-engine mask pass exists. Sum-of-exp is a ones-matmul
    (value 1/16: the fp8 quantize scale rides the reciprocal) into its
    own PSUM bank. PV/sum-exp lag the score group by one (software
    pipeline) and output-projection chunks fill exp-latency gaps.
  - PSUM banks are packed: phase-1 K0/K1, Q0/Q1, Vm0/m1, and ssq/s_bc
    share four banks (accumulation groups sharing a bank are strictly
    sequential - PSUM pending-zero is bank granular). The other four
    banks host the attention pools for the whole kernel, letting
    attention for query blocks 0-2 overlap the QKV phase (emitted after
    token blocks 1/3/5): its exp/chain stalls hide under dense QKV
    matmul work.
  - Output projection runs fp8 DoubleRow over the two head chunks
    (3-term hi/lo) producing natural-orientation [tok, hid] fp16 output
    at 1024x scale (the host folds the descale into its reduction);
    evictions are DVE with a 1-in-8 ACT share, the drain tail
    alternates engines and splits its DMA.
"""

import numpy as np
import ml_dtypes

from contextlib import ExitStack

import concourse.bacc as bacc
import concourse.mybir as mybir
import concourse.tile as tile
from concourse import bass_utils

# Problem shapes (hardcoded per contract)
S = 2048          # sequence length
H = 2048          # hidden
NH = 16           # heads
D = 128           # head dim
EPS = 1e-5
N_CORES = 8
HPC = NH // N_CORES        # heads per core = 2
FPC = 3 * D * HPC          # wqkv features per core = 768
CPC = D * HPC              # attn dims (wo input cols) per core = 256

TB = 256                   # token block width (phase 1)
NTB = S // TB              # 8
NP = 8                     # DoubleRow k-tile pairs over H (2048/256)
QB = 512                   # query block width (phase 2)
NKB = S // 128             # 16 key 128-blocks

SX = 16.0                  # fp8 scale for x and attn values
SW = 64.0                  # fp8 scale for wqkv and wo
DESCALE = 1.0 / (SX * SW)  # 1/1024
SQB = 32.0                 # Q/K eviction scale denominator: qk carry s/SQB
SQRT_D = float(np.sqrt(D))
# qkT carries (SX*SW/SQB)*s*Q~, so logits = score_psum/((SX*SW/SQB)^2 sqrt(D))
EXP_SCALE = 1.0 / ((SX * SW / SQB) ** 2 * SQRT_D)

f32 = mybir.dt.float32
f32r = mybir.dt.float32r
f16 = mybir.dt.float16
f8 = mybir.dt.float8e4
DR = mybir.MatmulPerfMode.DoubleRow
MULT = mybir.AluOpType.mult
SUB = mybir.AluOpType.subtract
EXP = mybir.ActivationFunctionType.Exp
SQRT = mybir.ActivationFunctionType.Sqrt
SQUARE = mybir.ActivationFunctionType.Square
COPY = mybir.ActivationFunctionType.Copy

E4M3 = ml_dtypes.float8_e4m3

_CACHED_NC = None


def _build():
    nc = bacc.Bacc("TRN2", target_bir_lowering=False, debug=False,
                   num_devices=N_CORES)
    # x8: [tb, p, hilo, pair, two, t_rel] packed fp8 (hi and lo at x*SX scale)
    x8_d = nc.dram_tensor("x8", [NTB, 128, 2 * NP * 2 * TB], f8,
                          kind="ExternalInput").ap()
    # w8: [hilo, pair, p, two, f'] fp8, f' = [q0|k0|q1|k1|v0|v1] each 128;
    # hi block first so tb0's term-1 matmuls are fed in consumption order
    w8_d = nc.dram_tensor("w8", [2, NP, 128, 2 * FPC], f8,
                          kind="ExternalInput").ap()
    # wo8: [p, hilo, two(head), hid] fp8
    wo8_d2 = nc.dram_tensor("wo8", [128, 2 * 2 * H], f8,
                            kind="ExternalInput").ap()
    # fp16 consts: [triC(128) | eye(128) | eye64(128)]
    cst_d = nc.dram_tensor("cst16", [128, 384], f16, kind="ExternalInput").ap()
    # natural-orientation fp16 output [tok, hid], values at 1024x
    out_d = nc.dram_tensor("out", [S, H], f16, kind="ExternalOutput").ap()

    with tile.TileContext(nc) as tc:
        with ExitStack() as stack:
            ep = stack.enter_context
            const_pool = ep(tc.tile_pool(name="const", bufs=1))
            qk_pool = ep(tc.tile_pool(name="qk", bufs=1))
            v_pool = ep(tc.tile_pool(name="vsb", bufs=1))
            attn_pool = ep(tc.tile_pool(name="attn8", bufs=1))
            s_pool = ep(tc.tile_pool(name="svec", bufs=1))
            wo_pool = ep(tc.tile_pool(name="wop", bufs=1))
            exp_pool = ep(tc.tile_pool(name="exps", bufs=3))
            rse_pool = ep(tc.tile_pool(name="rse", bufs=2))
            a16_pool = ep(tc.tile_pool(name="a16", bufs=2))
            psum_s = ep(tc.tile_pool(name="ps_s", bufs=1, space="PSUM"))
            psum_po = ep(tc.tile_pool(name="ps_po", bufs=1, space="PSUM"))
            psum_pse = ep(tc.tile_pool(name="ps_pse", bufs=1, space="PSUM"))

            triC = const_pool.tile([128, 128], f16, tag="tri")
            eye16 = const_pool.tile([128, 128], f16, tag="eye")
            eye64 = const_pool.tile([128, 128], f16, tag="eye64")
            ones_c16 = const_pool.tile([128, 1], f16, tag="oc16")
            # sum-exp stationary: value 1/SX so recip yields SX/sumexp
            ones_se = const_pool.tile([128, 128], f16, tag="ose")
            row1 = const_pool.tile([1, 128], f16, tag="row1")
            eps_b = const_pool.tile([128, 1], f32, tag="eps")
            nc.gpsimd.memset(ones_c16[:], 1.0)
            nc.gpsimd.memset(ones_se[:], 1.0 / SX)
            nc.gpsimd.memset(row1[:], 1.0)
            # s chain emits SQB*sqrt(mean x^2 + eps): bias = eps*SQB^2
            nc.gpsimd.memset(eps_b[:], EPS * SQB * SQB)

            # tensors live across the whole kernel
            qkT = qk_pool.tile([128, 4, S], f16)      # [q0,k0,q1,k1] x S
            v_sb = v_pool.tile([128, NKB, CPC], f16)  # V natural, kt-chunked
            attn8h = attn_pool.tile([128, HPC, S], f8, tag="ah")
            attn8l = attn_pool.tile([128, HPC, S], f8, tag="al")
            s_bc = s_pool.tile([128, NTB, TB], f16)   # s/SQB bcast over parts
            sT = s_pool.tile([128, NKB], f32)         # s/SQB, t on parts
            sTv = s_pool.tile([128, NKB], f32)        # s/(SX*SW), t on parts
            wo8 = wo_pool.tile([128, 2, 2, H], f8)

            # ---- attention machinery (emitted interleaved with phase 1) ---
            fills = []
            stages = {}
            op_ctx = {}

            def outproj_chunk(qc, hb):
                if qc not in stages:
                    stages[qc] = op_ctx["stage"].tile(
                        [128, 4, QB], f16, tag="st", name=f"st{qc}")
                st = stages[qc]
                ps = op_ctx["psum"].tile([128, QB], f32, tag="op",
                                         name="opps")
                first = True
                for asel, wsel in ((0, 0), (1, 0), (0, 1)):
                    a8 = attn8h if asel == 0 else attn8l
                    nc.tensor.matmul(
                        ps[:], a8[:, :, qc * 128:(qc + 1) * 128],
                        wo8[:, wsel, :, hb * QB:(hb + 1) * QB],
                        perf_mode=DR, start=first,
                        stop=(asel, wsel) == (0, 1))
                    first = False
                # Pool cannot read PSUM; mostly DVE (ACT is exp-bound),
                # 1-in-8 on ACT, and the drain tail alternates engines.
                # The 1/1024 descale moves to the host's reduction.
                if ((qc >= 12 and hb % 2 == 0)
                        or (qc < 12 and (qc * 4 + hb) % 8 == 0)):
                    nc.scalar.copy(st[:, hb], ps[:])
                else:
                    nc.vector.tensor_copy(st[:, hb], ps[:])
                if qc == 15 and hb == 1:
                    nc.sync.dma_start(
                        out_d[qc * 128:(qc + 1) * 128, 0:2 * QB], st[:, 0:2])
                if hb == 3:
                    if qc == 15:
                        nc.sync.dma_start(
                            out_d[qc * 128:(qc + 1) * 128, 2 * QB:],
                            st[:, 2:4])
                    else:
                        nc.sync.dma_start(
                            out_d[qc * 128:(qc + 1) * 128, :], st[:])
                    del stages[qc]

            def emit_fills(n):
                for _ in range(min(n, len(fills))):
                    qc, hb = fills.pop(0)
                    outproj_chunk(qc, hb)

            def attn_head(qb, h):
                q_slot, k_slot = 2 * h, 2 * h + 1
                kb_hi = qb * 4 + 3
                po = psum_po.tile([128, QB], f32, tag="po", name="po")
                pse = psum_pse.tile([128, QB], f32, tag="pse", name="pse")

                def pv_group(g, es):
                    for kr in range(2):
                        kb = 2 * g + kr
                        j = kb - 4 * qb
                        lo = max(0, j) * 128
                        nc.tensor.matmul(
                            po[:, lo:], v_sb[:, kb, h * D:(h + 1) * D],
                            es[:, kr, lo:],
                            start=(kb == 0), stop=(kb == kb_hi))
                        nc.tensor.matmul(
                            pse[:, lo:], ones_se[:], es[:, kr, lo:],
                            start=(kb == 0), stop=(kb == kb_hi))

                es_prev = None
                prev_g = None
                for g in range(2 * (qb + 1)):
                    ps = psum_s.tile([128, 2, QB], f32, tag="ps", name="ps")
                    es = exp_pool.tile([128, 2, QB], f16, tag="es",
                                       name="es")
                    diag = g >= 2 * qb
                    for kr in range(2):
                        kb = 2 * g + kr
                        j = kb - 4 * qb
                        lo = max(0, j) * 128
                        nc.tensor.matmul(
                            ps[:, kr, lo:],
                            qkT[:, k_slot, kb * 128:(kb + 1) * 128],
                            qkT[:, q_slot, qb * QB + lo:(qb + 1) * QB],
                            start=True, stop=not diag)
                        if diag:
                            # causal mask folded into the score psum: adds
                            # -290k (64 * -4525) above the diagonal so exp
                            # underflows to zero - no mask op anywhere
                            nc.tensor.matmul(
                                ps[:, kr, lo:lo + 128], eye64[:], triC[:],
                                start=False, stop=True)
                    if diag:
                        for kr in range(2):
                            kb = 2 * g + kr
                            lo = (kb - 4 * qb) * 128
                            nc.scalar.activation(es[:, kr, lo:],
                                                 ps[:, kr, lo:], EXP,
                                                 scale=EXP_SCALE)
                    else:
                        nc.scalar.activation(es[:], ps[:], EXP,
                                             scale=EXP_SCALE)
                    if es_prev is not None:
                        pv_group(prev_g, es_prev)
                        emit_fills(2)
                    es_prev, prev_g = es, g
                # cover the final (diagonal) group's exp latency with
                # output-projection work before its PV runs
                emit_fills(3)
                pv_group(prev_g, es_prev)

                # A = SX * po / sumexp (pse holds sumexp/SX) then fp8 hi/lo
                # quantize; recip lands in SBUF so a16 reads only one PSUM
                rse = rse_pool.tile([128, QB], f32, tag="rse", name="rse")
                nc.vector.reciprocal_approx_fast(rse[:], pse[:])
                a16 = a16_pool.tile([128, QB], f16, tag="a16", name="a16")
                nc.vector.tensor_tensor(a16[:], po[:], rse[:], MULT)
                if (qb, h) == (3, 1):
                    # drain tail: quantize on the fast engines, in halves,
                    # so the first tail outproj chunks start sooner
                    for z in range(2):
                        zz = slice(qb * QB + z * 256, qb * QB + z * 256 + 256)
                        az = slice(z * 256, z * 256 + 256)
                        nc.scalar.copy(attn8h[:, h, zz], a16[:, az])
                        nc.vector.tensor_tensor(
                            attn8l[:, h, zz], a16[:, az],
                            attn8h[:, h, zz], SUB)
                else:
                    nc.gpsimd.tensor_copy(
                        attn8h[:, h, qb * QB:(qb + 1) * QB], a16[:])
                    nc.gpsimd.tensor_tensor(
                        attn8l[:, h, qb * QB:(qb + 1) * QB], a16[:],
                        attn8h[:, h, qb * QB:(qb + 1) * QB], SUB)

            # ---------------- Phase 1: RMSNorm stats + QKV projection ------
            # (attention for query blocks 0-2 is emitted after token blocks
            # 1/3/5 and executes under the dense QKV matmul stream)
            with ExitStack() as ph1_stack:
                ep1 = ph1_stack.enter_context
                wt_pool = ep1(tc.tile_pool(name="wt", bufs=1))
                xt_pool = ep1(tc.tile_pool(name="xt", bufs=2))
                sq_pool = ep1(tc.tile_pool(name="sq", bufs=2))
                ph1_pool = ep1(tc.tile_pool(name="ph1", bufs=2))
                psum_kk = ep1(tc.tile_pool(name="ps_kk", bufs=1, space="PSUM"))
                psum_qq = ep1(tc.tile_pool(name="ps_qq", bufs=1, space="PSUM"))
                psum_v = ep1(tc.tile_pool(name="ps_v", bufs=1, space="PSUM"))
                psum_ms = ep1(tc.tile_pool(name="ps_ms", bufs=1, space="PSUM"))

                # weights in consumption order: w-hi pairs chunked on the
                # ACT queue while x streams on the sync queue, then w-lo
                w8 = wt_pool.tile([128, 2, NP, 2, FPC], f8, tag="w8")
                xt_cur = xt_pool.tile([128, 2, NP, 2, TB], f8, tag="xt")
                half = NP // 2 * 2 * TB
                nc.sync.dma_start(
                    xt_cur[:, 0, 0:NP // 2],
                    x8_d[0, :, 0:half]
                    .rearrange("p (j two t) -> p j two t", j=NP // 2, two=2))
                for hl in range(2):
                    for j0 in range(0, NP, 2):
                        nc.scalar.dma_start(
                            w8[:, hl, j0:j0 + 2],
                            w8_d[hl, j0:j0 + 2]
                            .rearrange("j p (two f) -> p j two f", two=2))
                nc.sync.dma_start(
                    xt_cur[:, 0, NP // 2:],
                    x8_d[0, :, half:2 * half]
                    .rearrange("p (j two t) -> p j two t", j=NP // 2, two=2))
                nc.sync.dma_start(
                    xt_cur[:, 1],
                    x8_d[0, :, NP * 2 * TB:]
                    .rearrange("p (j two t) -> p j two t", j=NP, two=2))
                nc.sync.dma_start(triC[:], cst_d[:, 0:128])
                nc.sync.dma_start(eye16[:], cst_d[:, 128:256])
                nc.sync.dma_start(eye64[:], cst_d[:, 256:384])


                def squares(xt_tile, tag, first=False):
                    # squares of x-hi (scaled 16x): half ACT, half DVE;
                    # called one tb ahead of use so they never gate ssq.
                    # tb0's ACT half moves to DVE: the ACT sequencer is
                    # busy with w8 DGE configs during the ramp.
                    sq = sq_pool.tile([128, NP, 2, TB], f16, tag="sq",
                                      name=f"sq{tag}")
                    if first:
                        nc.vector.tensor_tensor(
                            sq[:, 0:NP // 2].rearrange("p a b c -> p (a b c)"),
                            xt_tile[:, 0, 0:NP // 2]
                            .rearrange("p a b c -> p (a b c)"),
                            xt_tile[:, 0, 0:NP // 2]
                            .rearrange("p a b c -> p (a b c)"), MULT)
                        nc.vector.tensor_tensor(
                            sq[:, NP // 2:].rearrange("p a b c -> p (a b c)"),
                            xt_tile[:, 0, NP // 2:]
                            .rearrange("p a b c -> p (a b c)"),
                            xt_tile[:, 0, NP // 2:]
                            .rearrange("p a b c -> p (a b c)"), MULT)
                        return sq
                    nc.scalar.activation(
                        sq[:, 0:NP // 2].rearrange("p a b c -> p (a b c)"),
                        xt_tile[:, 0, 0:NP // 2]
                        .rearrange("p a b c -> p (a b c)"), SQUARE)
                    nc.vector.tensor_tensor(
                        sq[:, NP // 2:].rearrange("p a b c -> p (a b c)"),
                        xt_tile[:, 0, NP // 2:]
                        .rearrange("p a b c -> p (a b c)"),
                        xt_tile[:, 0, NP // 2:]
                        .rearrange("p a b c -> p (a b c)"), MULT)
                    return sq

                sq_cur = squares(xt_cur, 0)

                for tb in range(NTB):
                    xt = xt_cur
                    sq = sq_cur
                    if tb + 1 < NTB:
                        xt_next = xt_pool.tile([128, 2, NP, 2, TB], f8,
                                               tag="xt")
                        nc.sync.dma_start(
                            xt_next[:],
                            x8_d[tb + 1].rearrange(
                                "p (hl j two t) -> p hl j two t", hl=2, j=NP,
                                two=2))

                    def qk_dr(fb, ps):
                        # 3-term hi/lo DoubleRow accumulation for one slot
                        first = True
                        for wsel, xsel in ((0, 0), (1, 0), (0, 1)):
                            for j in range(NP):
                                nc.tensor.matmul(
                                    ps, w8[:, wsel, j, :,
                                           fb * 128:(fb + 1) * 128],
                                    xt[:, xsel, j], perf_mode=DR,
                                    start=first,
                                    stop=(wsel, xsel, j) == (0, 1, NP - 1))
                                first = False

                    # K blocks share one PSUM bank (strictly sequential
                    # accumulation groups - pending-zero is bank-granular)
                    ps_k = psum_kk.tile([128, 2, TB], f32, tag="kk",
                                        name="ps_k")
                    qk_dr(1, ps_k[:, 0])

                    # per-token sum of squares: 1-col transposed-stationary,
                    # m-outer so the two column groups are sequential
                    ms = psum_ms.tile([128, QB], f32, tag="ms", name="ms")
                    for m in range(2):
                        for j in range(NP):
                            for two in range(2):
                                nc.tensor.matmul(
                                    ms[:, m:m + 1],
                                    sq[:, j, two, m * 128:(m + 1) * 128],
                                    ones_c16[:],
                                    start=(j == 0 and two == 0),
                                    stop=(j == NP - 1 and two == 1))

                    qk_dr(3, ps_k[:, 1])

                    # s/SQB = 1/(SQB*sqrt(mean x^2 + eps)); ms holds
                    # SX^2*ssq so scale by SQB^2/(SX^2*H). Runs on ACT/DVE
                    # under K block 1.
                    sqrt_t = ph1_pool.tile([128, 4], f32, tag="sqrt")
                    nc.scalar.activation(sqrt_t[:, 0:2], ms[:, 0:2], SQRT,
                                         bias=eps_b[:],
                                         scale=SQB * SQB / (SX * SX * H))
                    nc.vector.reciprocal_approx_fast(sT[:, 2 * tb:2 * tb + 2],
                                                     sqrt_t[:, 0:2])
                    # V-eviction scale s/(SX*SW), per-partition for ACT
                    nc.scalar.mul(sTv[:, 2 * tb:2 * tb + 2],
                                  sT[:, 2 * tb:2 * tb + 2], SQB * DESCALE)
                    s16 = ph1_pool.tile([128, 2], f16, tag="s16")
                    nc.vector.tensor_copy(s16[:], sT[:, 2 * tb:2 * tb + 2])
                    # transposes to partition-0 rows land in the ms bank
                    # (f16 cols 8:264 = f32 cols 4:132), then 1-partition
                    # broadcast matmuls into f32 cols 256:512
                    ms16 = ms.bitcast(f16)
                    for m in range(2):
                        nc.tensor.transpose(
                            ms16[0:1, 8 + m * 128:136 + m * 128],
                            s16[:, m:m + 1], eye16[:])
                    srow = ph1_pool.tile([1, 256], f16, tag="srow")
                    nc.vector.tensor_copy(srow[:], ms16[0:1, 8:264])

                    ps_q = psum_qq.tile([128, 2, TB], f32, tag="qq",
                                        name="ps_q")
                    qk_dr(0, ps_q[:, 0])
                    for m in range(2):
                        nc.tensor.matmul(
                            ms[:, 256 + m * 128:256 + (m + 1) * 128],
                            row1[:], srow[0:1, m * 128:(m + 1) * 128],
                            start=True, stop=True)
                    qk_dr(2, ps_q[:, 1])
                    nc.scalar.copy(s_bc[:, tb], ms[:, 256:512])
                    for slot, ps in ((1, ps_k[:, 0]), (3, ps_k[:, 1]),
                                     (0, ps_q[:, 0]), (2, ps_q[:, 1])):
                        nc.vector.tensor_tensor(
                            qkT[:, slot, tb * TB:(tb + 1) * TB], ps,
                            s_bc[:, tb], MULT)

                    # next tb's squares: queue behind this tb's chain ops,
                    # run during the V blocks
                    if tb + 1 < NTB:
                        sq_cur = squares(xt_next, (tb + 1) % 2)

                    # V blocks: out (t, dv); lhsT = x pairs, rhs = wv pairs;
                    # both m chunks share one PSUM bank (sequential groups)
                    ps_v = psum_v.tile([128, 2, CPC], f32, tag="vv",
                                       name="ps_v")
                    for m in range(2):
                        first = True
                        for xsel, wsel in ((0, 0), (0, 1), (1, 0)):
                            for j in range(NP):
                                nc.tensor.matmul(
                                    ps_v[:, m],
                                    xt[:, xsel, j, :, m * 128:(m + 1) * 128],
                                    w8[:, wsel, j, :, 4 * 128:6 * 128],
                                    perf_mode=DR, start=first,
                                    stop=(xsel, wsel, j) == (1, 0, NP - 1))
                                first = False
                        chunk = tb * 2 + m
                        # v = ps * s/(SX*SW) = V~ * s, on ACT (per-partition)
                        nc.scalar.activation(v_sb[:, chunk], ps_v[:, m],
                                             COPY,
                                             scale=sTv[:, chunk:chunk + 1])

                    if tb + 1 < NTB:
                        xt_cur = xt_next

                    if tb == 1:
                        nc.sync.dma_start(
                            wo8[:], wo8_d2.rearrange(
                                "p (hl two o) -> p hl two o", hl=2, two=2))
                    # overlap: attention for ready query blocks runs under
                    # the remaining QKV work
                    if tb in (1, 3, 5):
                        attn_head(tb // 2, 0)
                        attn_head(tb // 2, 1)

            # -------- Phase 2 tail: last attention block + output proj -----
            with ExitStack() as op_stack:
                ep2 = op_stack.enter_context
                out_pool = ep2(tc.tile_pool(name="ostage", bufs=4))
                psum_op = ep2(tc.tile_pool(name="ps_op", bufs=4,
                                           space="PSUM"))
                op_ctx["stage"] = out_pool
                op_ctx["psum"] = psum_op
                fills.extend((qb * 4 + qq, hb) for qb in range(3)
                             for qq in range(4) for hb in range(4))
                attn_head(3, 0)
                attn_head(3, 1)
                fills.extend((12 + qq, hb)
                             for qq in range(4) for hb in range(4))
                emit_fills(len(fills))
    nc.compile()
    return nc


def get_nc():
    global _CACHED_NC
    if _CACHED_NC is None:
        _CACHED_NC = _build()
    return _CACHED_NC


def _hilo(a, scale):
    hi = (a * scale).astype(E4M3)
    lo = (a * scale - hi.astype(np.float32)).astype(E4M3)
    return hi, lo


def make_in_maps(x, wqkv, wo):
    x = np.asarray(x, dtype=np.float32)
    wqkv = np.asarray(wqkv, dtype=np.float32)
    wo = np.asarray(wo, dtype=np.float32)

    # x8: [tb, p, hilo, pair, two, t] from xT[h = pair*256 + two*128 + p, t]
    xh, xl = _hilo(np.ascontiguousarray(x.T), SX)
    x8 = np.stack([a.reshape(NP, 2, 128, NTB, TB).transpose(3, 2, 0, 1, 4)
                   for a in (xh, xl)], axis=2)
    x8 = np.ascontiguousarray(x8.reshape(NTB, 128, 2 * NP * 2 * TB))

    cst = np.concatenate(
        [np.tril(np.ones((128, 128), np.float32), -1).astype(np.float16)
         * np.float16(-4525.0),
         np.eye(128, dtype=np.float16),
         np.eye(128, dtype=np.float16) * np.float16(64.0)], axis=1)

    in_maps = []
    for c in range(N_CORES):
        wT = wqkv[c * FPC:(c + 1) * FPC].T            # [2048h, 768f]
        # feature permute to [q0|k0|q1|k1|v0|v1]
        perm = np.r_[0:128, 128:256, 384:512, 512:640, 256:384, 640:768]
        wT = np.ascontiguousarray(wT[:, perm])
        wh, wl = _hilo(wT, SW)
        w8 = np.stack([a.reshape(NP, 2, 128, FPC).transpose(0, 2, 1, 3)
                       for a in (wh, wl)], axis=0)    # [hl, j, p, two, f]
        w8 = np.ascontiguousarray(w8.reshape(2, NP, 128, 2 * FPC))

        woT = np.ascontiguousarray(wo[:, c * CPC:(c + 1) * CPC].T)  # [256,2048]
        oh, ol = _hilo(woT, SW)
        wo8 = np.stack([a.reshape(2, 128, H).transpose(1, 0, 2)
                        for a in (oh, ol)], axis=1)   # [p, hl, two, o]
        wo8 = np.ascontiguousarray(wo8.reshape(128, 2 * 2 * H))

        in_maps.append({"x8": x8, "w8": w8, "wo8": wo8, "cst16": cst})
    return in_maps


def kernel(x, wqkv, wo):
    nc = get_nc()
    in_maps = make_in_maps(x, wqkv, wo)
    res = None
    for attempt in range(4):
        try:
            res = bass_utils.run_bass_kernel_spmd(
                nc, in_maps, core_ids=list(range(N_CORES)))
            break
        except Exception:
            # transient NRT device wedges have been observed; they recover
            # after a short quiescent period, so back off before retrying
            if attempt == 3:
                raise
            import time
            time.sleep(20 * (attempt + 1))
    out = np.zeros((S, H), dtype=np.float32)
    for c in range(N_CORES):
        out += res.results[c]["out"].astype(np.float32)
    # device output carries the fp8 product scale; descale once here
    return out * np.float32(DESCALE)


# revision 66
# speedup vs baseline: 1.0296x; 1.0296x over previous
"""Trainium2 Bass kernel for nn_Attention_30305289240928.

Single-layer causal attention with RMSNorm prologue:
    xn = x * rsqrt(mean(x^2) + eps)           (RMSNorm, no weight)
    qkv = xn @ wqkv.T  -> per-head q, k, v    (16 heads, head_dim 128)
    out = softmax(causal(q k^T / sqrt(128))) v, concat heads, @ wo.T

Sharding: head-parallel tensor parallel over 8 NeuronCores.
Core c owns heads 2c, 2c+1 (wqkv rows c*768:(c+1)*768) and the matching
wo input-columns c*256:(c+1)*256. Each core computes a full-shape partial
of the output projection; the host sums the 8 partials.

Device-side design (v3, fp8/fp16 mixed precision, fused phases):
  - QKV projection runs on fp8e4m3 DoubleRow matmuls (2 k-tiles per
    instruction at 0.5 cycles/col = 4x fp32r element throughput). Inputs
    are host-quantized into scaled hi/lo pairs (x*16, w*64, hi and lo at
    the same scale); the product uses the 3-term correction
    Wh@Xh + Wl@Xh + Wh@Xl (dropped Wl@Xl term ~1e-3 relative).
  - RMSNorm: squares of x-hi on ACT+DVE one block ahead; per-token sums
    via 1-column transposed-stationary matmuls (~1 PE cycle each). The
    broadcast s_bc (= s/32, folding the fp8 descale) is built with a
    tiny fp16 transpose plus 1-partition broadcast matmuls. Both Q and
    K evict with the s_bc multiply, so exp's scale is a constant and
    exp batches over multi-block score groups.
  - Attention is fp16 (fp32r-grade mantissa, full PE rate at any N,
    enabling exact-causal column trimming). Scores are computed
    transposed, S.T[kt, qt]; causal masking is folded into the score
    PSUM accumulation as a -290k rank-128 add (exp underflows to 0), so
    no vector-engine mask pass exists. Sum-of-exp is a ones-matmul
    (value 1/16: the fp8 quantize scale rides the reciprocal) into its
    own PSUM bank. PV/sum-exp lag the score group by one (software
    pipeline) and output-projection chunks fill exp-latency gaps.
  - PSUM banks are packed: phase-1 K0/K1, Q0/Q1, Vm0/m1, and ssq/s_bc
    share four banks (accumulation groups sharing a bank are strictly
    sequential - PSUM pending-zero is bank granular). The other four
    banks host the attention pools for the whole kernel, letting
    attention for query blocks 0-2 overlap the QKV phase (emitted after
    token blocks 1/3/5): its exp/chain stalls hide under dense QKV
    matmul work.
  - Output projection runs fp8 DoubleRow over the two head chunks
    (3-term hi/lo) producing natural-orientation [tok, hid] fp16 output
    at 1024x scale (the host folds the descale into its reduction);
    evictions are DVE with a 1-in-8 ACT share, the drain tail
    alternates engines and splits its DMA.
"""

import numpy as np
import ml_dtypes

from contextlib import ExitStack

import concourse.bacc as bacc
import concourse.mybir as mybir
import concourse.tile as tile
from concourse import bass_utils

# Problem shapes (hardcoded per contract)
S = 2048          # sequence length
H = 2048          # hidden
NH = 16           # heads
D = 128           # head dim
EPS = 1e-5
N_CORES = 8
HPC = NH // N_CORES        # heads per core = 2
FPC = 3 * D * HPC          # wqkv features per core = 768
CPC = D * HPC              # attn dims (wo input cols) per core = 256

TB = 256                   # token block width (phase 1)
NTB = S // TB              # 8
NP = 8                     # DoubleRow k-tile pairs over H (2048/256)
QB = 512                   # query block width (phase 2)
NKB = S // 128             # 16 key 128-blocks

SX = 16.0                  # fp8 scale for x and attn values
SW = 64.0                  # fp8 scale for wqkv and wo
DESCALE = 1.0 / (SX * SW)  # 1/1024
SQB = 32.0                 # Q/K eviction scale denominator: qk carry s/SQB
SQRT_D = float(np.sqrt(D))
# qkT carries (SX*SW/SQB)*s*Q~, so logits = score_psum/((SX*SW/SQB)^2 sqrt(D))
EXP_SCALE = 1.0 / ((SX * SW / SQB) ** 2 * SQRT_D)

f32 = mybir.dt.float32
f32r = mybir.dt.float32r
f16 = mybir.dt.float16
f8 = mybir.dt.float8e4
DR = mybir.MatmulPerfMode.DoubleRow
MULT = mybir.AluOpType.mult
SUB = mybir.AluOpType.subtract
EXP = mybir.ActivationFunctionType.Exp
SQRT = mybir.ActivationFunctionType.Sqrt
SQUARE = mybir.ActivationFunctionType.Square
COPY = mybir.ActivationFunctionType.Copy

E4M3 = ml_dtypes.float8_e4m3

_CACHED_NC = None


def _build():
    nc = bacc.Bacc("TRN2", target_bir_lowering=False, debug=False,
                   num_devices=N_CORES)
    # x8: [tb, p, hilo, pair, two, t_rel] packed fp8 (hi and lo at x*SX scale)
    x8_d = nc.dram_tensor("x8", [NTB, 128, 2 * NP * 2 * TB], f8,
                          kind="ExternalInput").ap()
    # w8: [hilo, pair, p, two, f'] fp8, f' = [q0|k0|q1|k1|v0|v1] each 128;
    # hi block first so tb0's term-1 matmuls are fed in consumption order
    w8_d = nc.dram_tensor("w8", [2, NP, 128, 2 * FPC], f8,
                          kind="ExternalInput").ap()
    # wo8: [p, hilo, two(head), hid] fp8
    wo8_d2 = nc.dram_tensor("wo8", [128, 2 * 2 * H], f8,
                            kind="ExternalInput").ap()
    # fp16 consts: [triC(128) | eye(128) | eye64(128)]
    cst_d = nc.dram_tensor("cst16", [128, 384], f16, kind="ExternalInput").ap()
    # natural-orientation fp16 output [tok, hid], values at 1024x
    out_d = nc.dram_tensor("out", [S, H], f16, kind="ExternalOutput").ap()

    with tile.TileContext(nc) as tc:
        with ExitStack() as stack:
            ep = stack.enter_context
            const_pool = ep(tc.tile_pool(name="const", bufs=1))
            qk_pool = ep(tc.tile_pool(name="qk", bufs=1))
            v_pool = ep(tc.tile_pool(name="vsb", bufs=1))
            attn_pool = ep(tc.tile_pool(name="attn8", bufs=1))
            s_pool = ep(tc.tile_pool(name="svec", bufs=1))
            wo_pool = ep(tc.tile_pool(name="wop", bufs=1))
            exp_pool = ep(tc.tile_pool(name="exps", bufs=3))
            rse_pool = ep(tc.tile_pool(name="rse", bufs=2))
            a16_pool = ep(tc.tile_pool(name="a16", bufs=2))
            psum_s = ep(tc.tile_pool(name="ps_s", bufs=1, space="PSUM"))
            psum_po = ep(tc.tile_pool(name="ps_po", bufs=1, space="PSUM"))
            psum_pse = ep(tc.tile_pool(name="ps_pse", bufs=1, space="PSUM"))

            triC = const_pool.tile([128, 128], f16, tag="tri")
            eye16 = const_pool.tile([128, 128], f16, tag="eye")
            eye64 = const_pool.tile([128, 128], f16, tag="eye64")
            ones_c16 = const_pool.tile([128, 1], f16, tag="oc16")
            # sum-exp stationary: value 1/SX so recip yields SX/sumexp
            ones_se = const_pool.tile([128, 128], f16, tag="ose")
            row1 = const_pool.tile([1, 128], f16, tag="row1")
            eps_b = const_pool.tile([128, 1], f32, tag="eps")
            nc.gpsimd.memset(ones_c16[:], 1.0)
            nc.gpsimd.memset(ones_se[:], 1.0 / SX)
            nc.gpsimd.memset(row1[:], 1.0)
            # s chain emits SQB*sqrt(mean x^2 + eps): bias = eps*SQB^2
            nc.gpsimd.memset(eps_b[:], EPS * SQB * SQB)

            # tensors live across the whole kernel
            qkT = qk_pool.tile([128, 4, S], f16)      # [q0,k0,q1,k1] x S
            v_sb = v_pool.tile([128, NKB, CPC], f16)  # V natural, kt-chunked
            attn8h = attn_pool.tile([128, HPC, S], f8, tag="ah")
            attn8l = attn_pool.tile([128, HPC, S], f8, tag="al")
            s_bc = s_pool.tile([128, NTB, TB], f16)   # s/SQB bcast over parts
            sT = s_pool.tile([128, NKB], f32)         # s/SQB, t on parts
            sTv = s_pool.tile([128, NKB], f32)        # s/(SX*SW), t on parts
            wo8 = wo_pool.tile([128, 2, 2, H], f8)

            # ---- attention machinery (emitted interleaved with phase 1) ---
            fills = []
            stages = {}
            op_ctx = {}

            def outproj_chunk(qc, hb):
                if qc not in stages:
                    stages[qc] = op_ctx["stage"].tile(
                        [128, 4, QB], f16, tag="st", name=f"st{qc}")
                st = stages[qc]
                ps = op_ctx["psum"].tile([128, QB], f32, tag="op",
                                         name="opps")
                first = True
                for asel, wsel in ((0, 0), (1, 0), (0, 1)):
                    a8 = attn8h if asel == 0 else attn8l
                    nc.tensor.matmul(
                        ps[:], a8[:, :, qc * 128:(qc + 1) * 128],
                        wo8[:, wsel, :, hb * QB:(hb + 1) * QB],
                        perf_mode=DR, start=first,
                        stop=(asel, wsel) == (0, 1))
                    first = False
                # Pool cannot read PSUM; mostly DVE (ACT is exp-bound),
                # 1-in-8 on ACT, and the drain tail alternates engines.
                # The 1/1024 descale moves to the host's reduction.
                if ((qc >= 12 and hb % 2 == 0)
                        or (qc < 12 and (qc * 4 + hb) % 8 == 0)):
                    nc.scalar.copy(st[:, hb], ps[:])
                else:
                    nc.vector.tensor_copy(st[:, hb], ps[:])
                if qc == 15 and hb == 1:
                    nc.sync.dma_start(
                        out_d[qc * 128:(qc + 1) * 128, 0:2 * QB], st[:, 0:2])
                if hb == 3:
                    if qc == 15:
                        nc.sync.dma_start(
                            out_d[qc * 128:(qc + 1) * 128, 2 * QB:],
                            st[:, 2:4])
                    else:
                        nc.sync.dma_start(
                            out_d[qc * 128:(qc + 1) * 128, :], st[:])
                    del stages[qc]

            def emit_fills(n):
                for _ in range(min(n, len(fills))):
                    qc, hb = fills.pop(0)
                    outproj_chunk(qc, hb)

            def attn_head(qb, h):
                q_slot, k_slot = 2 * h, 2 * h + 1
                kb_hi = qb * 4 + 3
                po = psum_po.tile([128, QB], f32, tag="po", name="po")
                pse = psum_pse.tile([128, QB], f32, tag="pse", name="pse")

                def pv_group(g, es):
                    for kr in range(2):
                        kb = 2 * g + kr
                        j = kb - 4 * qb
                        lo = max(0, j) * 128
                        nc.tensor.matmul(
                            po[:, lo:], v_sb[:, kb, h * D:(h + 1) * D],
                            es[:, kr, lo:],
                            start=(kb == 0), stop=(kb == kb_hi))
                        nc.tensor.matmul(
                            pse[:, lo:], ones_se[:], es[:, kr, lo:],
                            start=(kb == 0), stop=(kb == kb_hi))

                es_prev = None
                prev_g = None
                for g in range(2 * (qb + 1)):
                    ps = psum_s.tile([128, 2, QB], f32, tag="ps", name="ps")
                    es = exp_pool.tile([128, 2, QB], f16, tag="es",
                                       name="es")
                    diag = g >= 2 * qb
                    for kr in range(2):
                        kb = 2 * g + kr
                        j = kb - 4 * qb
                        lo = max(0, j) * 128
                        nc.tensor.matmul(
                            ps[:, kr, lo:],
                            qkT[:, k_slot, kb * 128:(kb + 1) * 128],
                            qkT[:, q_slot, qb * QB + lo:(qb + 1) * QB],
                            start=True, stop=not diag)
                        if diag:
                            # causal mask folded into the score psum: adds
                            # -290k (64 * -4525) above the diagonal so exp
                            # underflows to zero - no mask op anywhere
                            nc.tensor.matmul(
                                ps[:, kr, lo:lo + 128], eye64[:], triC[:],
                                start=False, stop=True)
                    if diag:
                        for kr in range(2):
                            kb = 2 * g + kr
                            lo = (kb - 4 * qb) * 128
                            nc.scalar.activation(es[:, kr, lo:],
                                                 ps[:, kr, lo:], EXP,
                                                 scale=EXP_SCALE)
                    else:
                        nc.scalar.activation(es[:], ps[:], EXP,
                                             scale=EXP_SCALE)
                    if es_prev is not None:
                        pv_group(prev_g, es_prev)
                        emit_fills(2)
                    es_prev, prev_g = es, g
                # cover the final (diagonal) group's exp latency with
                # output-projection work before its PV runs
                emit_fills(3)
                pv_group(prev_g, es_prev)

                # A = SX * po / sumexp (pse holds sumexp/SX) then fp8 hi/lo
                # quantize; recip lands in SBUF so a16 reads only one PSUM
                rse = rse_pool.tile([128, QB], f32, tag="rse", name="rse")
                nc.vector.reciprocal_approx_fast(rse[:], pse[:])
                a16 = a16_pool.tile([128, QB], f16, tag="a16", name="a16")
                nc.vector.tensor_tensor(a16[:], po[:], rse[:], MULT)
                if (qb, h) == (3, 1):
                    # drain tail: quantize on the fast engines
                    nc.scalar.copy(
                        attn8h[:, h, qb * QB:(qb + 1) * QB], a16[:])
                    nc.vector.tensor_tensor(
                        attn8l[:, h, qb * QB:(qb + 1) * QB], a16[:],
                        attn8h[:, h, qb * QB:(qb + 1) * QB], SUB)
                else:
                    nc.gpsimd.tensor_copy(
                        attn8h[:, h, qb * QB:(qb + 1) * QB], a16[:])
                    nc.gpsimd.tensor_tensor(
                        attn8l[:, h, qb * QB:(qb + 1) * QB], a16[:],
                        attn8h[:, h, qb * QB:(qb + 1) * QB], SUB)

            # ---------------- Phase 1: RMSNorm stats + QKV projection ------
            # (attention for query blocks 0-2 is emitted after token blocks
            # 1/3/5 and executes under the dense QKV matmul stream)
            with ExitStack() as ph1_stack:
                ep1 = ph1_stack.enter_context
                wt_pool = ep1(tc.tile_pool(name="wt", bufs=1))
                xt_pool = ep1(tc.tile_pool(name="xt", bufs=2))
                sq_pool = ep1(tc.tile_pool(name="sq", bufs=2))
                ph1_pool = ep1(tc.tile_pool(name="ph1", bufs=2))
                psum_kk = ep1(tc.tile_pool(name="ps_kk", bufs=1, space="PSUM"))
                psum_qq = ep1(tc.tile_pool(name="ps_qq", bufs=1, space="PSUM"))
                psum_v = ep1(tc.tile_pool(name="ps_v", bufs=1, space="PSUM"))
                psum_ms = ep1(tc.tile_pool(name="ps_ms", bufs=1, space="PSUM"))

                # weights in consumption order: w-hi pairs chunked on the
                # ACT queue while x streams on the sync queue, then w-lo
                w8 = wt_pool.tile([128, 2, NP, 2, FPC], f8, tag="w8")
                xt_cur = xt_pool.tile([128, 2, NP, 2, TB], f8, tag="xt")
                half = NP // 2 * 2 * TB
                def wdma(hl, j0):
                    nc.sync.dma_start(
                        w8[:, hl, j0:j0 + 2],
                        w8_d[hl, j0:j0 + 2]
                        .rearrange("j p (two f) -> p j two f", two=2))

                nc.sync.dma_start(
                    xt_cur[:, 0, 0:NP // 2],
                    x8_d[0, :, 0:half]
                    .rearrange("p (j two t) -> p j two t", j=NP // 2, two=2))
                wdma(0, 0)
                wdma(0, 2)
                nc.sync.dma_start(
                    xt_cur[:, 0, NP // 2:],
                    x8_d[0, :, half:2 * half]
                    .rearrange("p (j two t) -> p j two t", j=NP // 2, two=2))
                wdma(0, 4)
                wdma(0, 6)
                wdma(1, 0)
                wdma(1, 2)
                nc.sync.dma_start(
                    xt_cur[:, 1],
                    x8_d[0, :, NP * 2 * TB:]
                    .rearrange("p (j two t) -> p j two t", j=NP, two=2))
                wdma(1, 4)
                wdma(1, 6)
                nc.sync.dma_start(triC[:], cst_d[:, 0:128])
                nc.sync.dma_start(eye16[:], cst_d[:, 128:256])
                nc.sync.dma_start(eye64[:], cst_d[:, 256:384])


                def squares(xt_tile, tag, first=False):
                    # squares of x-hi (scaled 16x): half ACT, half DVE;
                    # called one tb ahead of use so they never gate ssq.
                    # tb0's ACT half moves to DVE: the ACT sequencer is
                    # busy with w8 DGE configs during the ramp.
                    sq = sq_pool.tile([128, NP, 2, TB], f16, tag="sq",
                                      name=f"sq{tag}")
                    if first:
                        nc.vector.tensor_tensor(
                            sq[:, 0:NP // 2].rearrange("p a b c -> p (a b c)"),
                            xt_tile[:, 0, 0:NP // 2]
                            .rearrange("p a b c -> p (a b c)"),
                            xt_tile[:, 0, 0:NP // 2]
                            .rearrange("p a b c -> p (a b c)"), MULT)
                        nc.vector.tensor_tensor(
                            sq[:, NP // 2:].rearrange("p a b c -> p (a b c)"),
                            xt_tile[:, 0, NP // 2:]
                            .rearrange("p a b c -> p (a b c)"),
                            xt_tile[:, 0, NP // 2:]
                            .rearrange("p a b c -> p (a b c)"), MULT)
                        return sq
                    nc.scalar.activation(
                        sq[:, 0:NP // 2].rearrange("p a b c -> p (a b c)"),
                        xt_tile[:, 0, 0:NP // 2]
                        .rearrange("p a b c -> p (a b c)"), SQUARE)
                    nc.vector.tensor_tensor(
                        sq[:, NP // 2:].rearrange("p a b c -> p (a b c)"),
                        xt_tile[:, 0, NP // 2:]
                        .rearrange("p a b c -> p (a b c)"),
                        xt_tile[:, 0, NP // 2:]
                        .rearrange("p a b c -> p (a b c)"), MULT)
                    return sq

                sq_cur = squares(xt_cur, 0)

                for tb in range(NTB):
                    xt = xt_cur
                    sq = sq_cur
                    if tb + 1 < NTB:
                        xt_next = xt_pool.tile([128, 2, NP, 2, TB], f8,
                                               tag="xt")
                        nc.sync.dma_start(
                            xt_next[:],
                            x8_d[tb + 1].rearrange(
                                "p (hl j two t) -> p hl j two t", hl=2, j=NP,
                                two=2))

                    def qk_dr(fb, ps):
                        # 3-term hi/lo DoubleRow accumulation for one slot
                        first = True
                        for wsel, xsel in ((0, 0), (1, 0), (0, 1)):
                            for j in range(NP):
                                nc.tensor.matmul(
                                    ps, w8[:, wsel, j, :,
                                           fb * 128:(fb + 1) * 128],
                                    xt[:, xsel, j], perf_mode=DR,
                                    start=first,
                                    stop=(wsel, xsel, j) == (0, 1, NP - 1))
                                first = False

                    # K blocks share one PSUM bank (strictly sequential
                    # accumulation groups - pending-zero is bank-granular)
                    ps_k = psum_kk.tile([128, 2, TB], f32, tag="kk",
                                        name="ps_k")
                    qk_dr(1, ps_k[:, 0])

                    # per-token sum of squares: 1-col transposed-stationary,
                    # m-outer so the two column groups are sequential
                    ms = psum_ms.tile([128, QB], f32, tag="ms", name="ms")
                    for m in range(2):
                        for j in range(NP):
                            for two in range(2):
                                nc.tensor.matmul(
                                    ms[:, m:m + 1],
                                    sq[:, j, two, m * 128:(m + 1) * 128],
                                    ones_c16[:],
                                    start=(j == 0 and two == 0),
                                    stop=(j == NP - 1 and two == 1))

                    qk_dr(3, ps_k[:, 1])

                    # s/SQB = 1/(SQB*sqrt(mean x^2 + eps)); ms holds
                    # SX^2*ssq so scale by SQB^2/(SX^2*H). Runs on ACT/DVE
                    # under K block 1.
                    sqrt_t = ph1_pool.tile([128, 4], f32, tag="sqrt")
                    nc.scalar.activation(sqrt_t[:, 0:2], ms[:, 0:2], SQRT,
                                         bias=eps_b[:],
                                         scale=SQB * SQB / (SX * SX * H))
                    nc.vector.reciprocal_approx_fast(sT[:, 2 * tb:2 * tb + 2],
                                                     sqrt_t[:, 0:2])
                    # V-eviction scale s/(SX*SW), per-partition for ACT
                    nc.scalar.mul(sTv[:, 2 * tb:2 * tb + 2],
                                  sT[:, 2 * tb:2 * tb + 2], SQB * DESCALE)
                    s16 = ph1_pool.tile([128, 2], f16, tag="s16")
                    nc.vector.tensor_copy(s16[:], sT[:, 2 * tb:2 * tb + 2])
                    # transposes to partition-0 rows land in the ms bank
                    # (f16 cols 8:264 = f32 cols 4:132), then 1-partition
                    # broadcast matmuls into f32 cols 256:512
                    ms16 = ms.bitcast(f16)
                    for m in range(2):
                        nc.tensor.transpose(
                            ms16[0:1, 8 + m * 128:136 + m * 128],
                            s16[:, m:m + 1], eye16[:])
                    srow = ph1_pool.tile([1, 256], f16, tag="srow")
                    nc.vector.tensor_copy(srow[:], ms16[0:1, 8:264])

                    ps_q = psum_qq.tile([128, 2, TB], f32, tag="qq",
                                        name="ps_q")
                    qk_dr(0, ps_q[:, 0])
                    for m in range(2):
                        nc.tensor.matmul(
                            ms[:, 256 + m * 128:256 + (m + 1) * 128],
                            row1[:], srow[0:1, m * 128:(m + 1) * 128],
                            start=True, stop=True)
                    qk_dr(2, ps_q[:, 1])
                    nc.scalar.copy(s_bc[:, tb], ms[:, 256:512])
                    for slot, ps in ((1, ps_k[:, 0]), (3, ps_k[:, 1]),
                                     (0, ps_q[:, 0]), (2, ps_q[:, 1])):
                        nc.vector.tensor_tensor(
                            qkT[:, slot, tb * TB:(tb + 1) * TB], ps,
                            s_bc[:, tb], MULT)

                    # next tb's squares: queue behind this tb's chain ops,
                    # run during the V blocks
                    if tb + 1 < NTB:
                        sq_cur = squares(xt_next, (tb + 1) % 2)

                    # V blocks: out (t, dv); lhsT = x pairs, rhs = wv pairs;
                    # both m chunks share one PSUM bank (sequential groups)
                    ps_v = psum_v.tile([128, 2, CPC], f32, tag="vv",
                                       name="ps_v")
                    for m in range(2):
                        first = True
                        for xsel, wsel in ((0, 0), (0, 1), (1, 0)):
                            for j in range(NP):
                                nc.tensor.matmul(
                                    ps_v[:, m],
                                    xt[:, xsel, j, :, m * 128:(m + 1) * 128],
                                    w8[:, wsel, j, :, 4 * 128:6 * 128],
                                    perf_mode=DR, start=first,
                                    stop=(xsel, wsel, j) == (1, 0, NP - 1))
                                first = False
                        chunk = tb * 2 + m
                        # v = ps * s/(SX*SW) = V~ * s, on ACT (per-partition)
                        nc.scalar.activation(v_sb[:, chunk], ps_v[:, m],
                                             COPY,
                                             scale=sTv[:, chunk:chunk + 1])

                    if tb + 1 < NTB:
                        xt_cur = xt_next

                    if tb == 1:
                        nc.sync.dma_start(
                            wo8[:], wo8_d2.rearrange(
                                "p (hl two o) -> p hl two o", hl=2, two=2))
                    # overlap: attention for ready query blocks runs under
                    # the remaining QKV work
                    if tb in (1, 3, 5):
                        attn_head(tb // 2, 0)
                        attn_head(tb // 2, 1)

            # -------- Phase 2 tail: last attention block + output proj -----
            with ExitStack() as op_stack:
                ep2 = op_stack.enter_context
                out_pool = ep2(tc.tile_pool(name="ostage", bufs=4))
                psum_op = ep2(tc.tile_pool(name="ps_op", bufs=4,
                                           space="PSUM"))
                op_ctx["stage"] = out_pool
                op_ctx["psum"] = psum_op
                fills.extend((qb * 4 + qq, hb) for qb in range(3)
                             for qq in range(4) for hb in range(4))
                attn_head(3, 0)
                attn_head(3, 1)
                fills.extend((12 + qq, hb)
                             for qq in range(4) for hb in range(4))
                emit_fills(len(fills))
    nc.compile()
    return nc


def get_nc():
    global _CACHED_NC
    if _CACHED_NC is None:
        _CACHED_NC = _build()
    return _CACHED_NC


def _hilo(a, scale):
    hi = (a * scale).astype(E4M3)
    lo = (a * scale - hi.astype(np.float32)).astype(E4M3)
    return hi, lo


def make_in_maps(x, wqkv, wo):
    x = np.asarray(x, dtype=np.float32)
    wqkv = np.asarray(wqkv, dtype=np.float32)
    wo = np.asarray(wo, dtype=np.float32)

    # x8: [tb, p, hilo, pair, two, t] from xT[h = pair*256 + two*128 + p, t]
    xh, xl = _hilo(np.ascontiguousarray(x.T), SX)
    x8 = np.stack([a.reshape(NP, 2, 128, NTB, TB).transpose(3, 2, 0, 1, 4)
                   for a in (xh, xl)], axis=2)
    x8 = np.ascontiguousarray(x8.reshape(NTB, 128, 2 * NP * 2 * TB))

    cst = np.concatenate(
        [np.tril(np.ones((128, 128), np.float32), -1).astype(np.float16)
         * np.float16(-4525.0),
         np.eye(128, dtype=np.float16),
         np.eye(128, dtype=np.float16) * np.float16(64.0)], axis=1)

    in_maps = []
    for c in range(N_CORES):
        wT = wqkv[c * FPC:(c + 1) * FPC].T            # [2048h, 768f]
        # feature permute to [q0|k0|q1|k1|v0|v1]
        perm = np.r_[0:128, 128:256, 384:512, 512:640, 256:384, 640:768]
        wT = np.ascontiguousarray(wT[:, perm])
        wh, wl = _hilo(wT, SW)
        w8 = np.stack([a.reshape(NP, 2, 128, FPC).transpose(0, 2, 1, 3)
                       for a in (wh, wl)], axis=0)    # [hl, j, p, two, f]
        w8 = np.ascontiguousarray(w8.reshape(2, NP, 128, 2 * FPC))

        woT = np.ascontiguousarray(wo[:, c * CPC:(c + 1) * CPC].T)  # [256,2048]
        oh, ol = _hilo(woT, SW)
        wo8 = np.stack([a.reshape(2, 128, H).transpose(1, 0, 2)
                        for a in (oh, ol)], axis=1)   # [p, hl, two, o]
        wo8 = np.ascontiguousarray(wo8.reshape(128, 2 * 2 * H))

        in_maps.append({"x8": x8, "w8": w8, "wo8": wo8, "cst16": cst})
    return in_maps


def kernel(x, wqkv, wo):
    nc = get_nc()
    in_maps = make_in_maps(x, wqkv, wo)
    res = None
    for attempt in range(4):
        try:
            res = bass_utils.run_bass_kernel_spmd(
                nc, in_maps, core_ids=list(range(N_CORES)))
            break
        except Exception:
            # transient NRT device wedges have been observed; they recover
            # after a short quiescent period, so back off before retrying
            if attempt == 3:
                raise
            import time
            time.sleep(20 * (attempt + 1))
    out = np.zeros((S, H), dtype=np.float32)
    for c in range(N_CORES):
        out += res.results[c]["out"].astype(np.float32)
    # device output carries the fp8 product scale; descale once here
    return out * np.float32(DESCALE)


# revision 67
# speedup vs baseline: 1.0532x; 1.0229x over previous
"""Trainium2 Bass kernel for nn_Attention_30305289240928.

Single-layer causal attention with RMSNorm prologue:
    xn = x * rsqrt(mean(x^2) + eps)           (RMSNorm, no weight)
    qkv = xn @ wqkv.T  -> per-head q, k, v    (16 heads, head_dim 128)
    out = softmax(causal(q k^T / sqrt(128))) v, concat heads, @ wo.T

Sharding: head-parallel tensor parallel over 8 NeuronCores.
Core c owns heads 2c, 2c+1 (wqkv rows c*768:(c+1)*768) and the matching
wo input-columns c*256:(c+1)*256. Each core computes a full-shape partial
of the output projection; the host sums the 8 partials.

Device-side design (v3, fp8/fp16 mixed precision, fused phases):
  - QKV projection runs on fp8e4m3 DoubleRow matmuls (2 k-tiles per
    instruction at 0.5 cycles/col = 4x fp32r element throughput). Inputs
    are host-quantized into scaled hi/lo pairs (x*16, w*64, hi and lo at
    the same scale); the product uses the 3-term correction
    Wh@Xh + Wl@Xh + Wh@Xl (dropped Wl@Xl term ~1e-3 relative).
  - RMSNorm: squares of x-hi on ACT+DVE one block ahead; per-token sums
    via 1-column transposed-stationary matmuls (~1 PE cycle each). The
    broadcast s_bc (= s/32, folding the fp8 descale) is built with a
    tiny fp16 transpose plus 1-partition broadcast matmuls. Both Q and
    K evict with the s_bc multiply, so exp's scale is a constant and
    exp batches over multi-block score groups.
  - Attention is fp16 (fp32r-grade mantissa, full PE rate at any N,
    enabling exact-causal column trimming). Scores are computed
    transposed, S.T[kt, qt]; causal masking is folded into the score
    PSUM accumulation as a -290k rank-128 add (exp underflows to 0), so
    no vector-engine mask pass exists. Sum-of-exp is a ones-matmul
    (value 1/16: the fp8 quantize scale rides the reciprocal) into its
    own PSUM bank. PV/sum-exp lag the score group by one (software
    pipeline) and output-projection chunks fill exp-latency gaps.
  - PSUM banks are packed: phase-1 K0/K1, Q0/Q1, Vm0/m1, and ssq/s_bc
    share four banks (accumulation groups sharing a bank are strictly
    sequential - PSUM pending-zero is bank granular). The other four
    banks host the attention pools for the whole kernel, letting
    attention for query blocks 0-2 overlap the QKV phase (emitted after
    token blocks 1/3/5): its exp/chain stalls hide under dense QKV
    matmul work.
  - Output projection runs fp8 DoubleRow over the two head chunks
    (3-term hi/lo) producing natural-orientation [tok, hid] fp16 output
    at 1024x scale (the host folds the descale into its reduction);
    evictions are DVE with a 1-in-8 ACT share, the drain tail
    alternates engines and splits its DMA.
"""

import numpy as np
import ml_dtypes

from contextlib import ExitStack

import concourse.bacc as bacc
import concourse.mybir as mybir
import concourse.tile as tile
from concourse import bass_utils

# Problem shapes (hardcoded per contract)
S = 2048          # sequence length
H = 2048          # hidden
NH = 16           # heads
D = 128           # head dim
EPS = 1e-5
N_CORES = 8
HPC = NH // N_CORES        # heads per core = 2
FPC = 3 * D * HPC          # wqkv features per core = 768
CPC = D * HPC              # attn dims (wo input cols) per core = 256

TB = 256                   # token block width (phase 1)
NTB = S // TB              # 8
NP = 8                     # DoubleRow k-tile pairs over H (2048/256)
QB = 512                   # query block width (phase 2)
NKB = S // 128             # 16 key 128-blocks

SX = 16.0                  # fp8 scale for x and attn values
SW = 64.0                  # fp8 scale for wqkv and wo
DESCALE = 1.0 / (SX * SW)  # 1/1024
SQB = 32.0                 # Q/K eviction scale denominator: qk carry s/SQB
SQRT_D = float(np.sqrt(D))
# qkT carries (SX*SW/SQB)*s*Q~, so logits = score_psum/((SX*SW/SQB)^2 sqrt(D))
EXP_SCALE = 1.0 / ((SX * SW / SQB) ** 2 * SQRT_D)

f32 = mybir.dt.float32
f32r = mybir.dt.float32r
f16 = mybir.dt.float16
f8 = mybir.dt.float8e4
DR = mybir.MatmulPerfMode.DoubleRow
MULT = mybir.AluOpType.mult
SUB = mybir.AluOpType.subtract
EXP = mybir.ActivationFunctionType.Exp
SQRT = mybir.ActivationFunctionType.Sqrt
SQUARE = mybir.ActivationFunctionType.Square
COPY = mybir.ActivationFunctionType.Copy

E4M3 = ml_dtypes.float8_e4m3

_CACHED_NC = None


def _build():
    nc = bacc.Bacc("TRN2", target_bir_lowering=False, debug=False,
                   num_devices=N_CORES)
    # x8: [tb, p, hilo, pair, two, t_rel] packed fp8 (hi and lo at x*SX scale)
    x8_d = nc.dram_tensor("x8", [NTB, 128, 2 * NP * 2 * TB], f8,
                          kind="ExternalInput").ap()
    # w8: [hilo, pair, p, two, f'] fp8, f' = [q0|k0|q1|k1|v0|v1] each 128;
    # hi block first so tb0's term-1 matmuls are fed in consumption order
    w8_d = nc.dram_tensor("w8", [2, NP, 128, 2 * FPC], f8,
                          kind="ExternalInput").ap()
    # wo8: [p, hilo, two(head), hid] fp8
    wo8_d2 = nc.dram_tensor("wo8", [128, 2 * 2 * H], f8,
                            kind="ExternalInput").ap()
    # fp16 consts: [triC(128) | eye(128) | eye64(128)]
    cst_d = nc.dram_tensor("cst16", [128, 384], f16, kind="ExternalInput").ap()
    # natural-orientation fp16 output [tok, hid], values at 1024x
    out_d = nc.dram_tensor("out", [S, H], f16, kind="ExternalOutput").ap()

    with tile.TileContext(nc) as tc:
        with ExitStack() as stack:
            ep = stack.enter_context
            const_pool = ep(tc.tile_pool(name="const", bufs=1))
            qk_pool = ep(tc.tile_pool(name="qk", bufs=1))
            v_pool = ep(tc.tile_pool(name="vsb", bufs=1))
            attn_pool = ep(tc.tile_pool(name="attn8", bufs=1))
            s_pool = ep(tc.tile_pool(name="svec", bufs=1))
            wo_pool = ep(tc.tile_pool(name="wop", bufs=1))
            exp_pool = ep(tc.tile_pool(name="exps", bufs=3))
            rse_pool = ep(tc.tile_pool(name="rse", bufs=2))
            a16_pool = ep(tc.tile_pool(name="a16", bufs=2))
            psum_s = ep(tc.tile_pool(name="ps_s", bufs=1, space="PSUM"))
            psum_po = ep(tc.tile_pool(name="ps_po", bufs=1, space="PSUM"))
            psum_pse = ep(tc.tile_pool(name="ps_pse", bufs=1, space="PSUM"))

            triC = const_pool.tile([128, 128], f16, tag="tri")
            eye16 = const_pool.tile([128, 128], f16, tag="eye")
            eye64 = const_pool.tile([128, 128], f16, tag="eye64")
            ones_c16 = const_pool.tile([128, 1], f16, tag="oc16")
            # sum-exp stationary: value 1/SX so recip yields SX/sumexp
            ones_se = const_pool.tile([128, 128], f16, tag="ose")
            row1 = const_pool.tile([1, 128], f16, tag="row1")
            eps_b = const_pool.tile([128, 1], f32, tag="eps")
            nc.gpsimd.memset(ones_c16[:], 1.0)
            nc.gpsimd.memset(ones_se[:], 1.0 / SX)
            nc.gpsimd.memset(row1[:], 1.0)
            # s chain emits SQB*sqrt(mean x^2 + eps): bias = eps*SQB^2
            nc.gpsimd.memset(eps_b[:], EPS * SQB * SQB)

            # tensors live across the whole kernel
            qkT = qk_pool.tile([128, 4, S], f16)      # [q0,k0,q1,k1] x S
            v_sb = v_pool.tile([128, NKB, CPC], f16)  # V natural, kt-chunked
            attn8h = attn_pool.tile([128, HPC, S], f8, tag="ah")
            attn8l = attn_pool.tile([128, HPC, S], f8, tag="al")
            s_bc = s_pool.tile([128, NTB, TB], f16)   # s/SQB bcast over parts
            sT = s_pool.tile([128, NKB], f32)         # s/SQB, t on parts
            sTv = s_pool.tile([128, NKB], f32)        # s/(SX*SW), t on parts
            wo8 = wo_pool.tile([128, 2, 2, H], f8)

            # ---- attention machinery (emitted interleaved with phase 1) ---
            fills = []
            stages = {}
            op_ctx = {}

            def outproj_chunk(qc, hb):
                if qc not in stages:
                    stages[qc] = op_ctx["stage"].tile(
                        [128, 4, QB], f16, tag="st", name=f"st{qc}")
                st = stages[qc]
                ps = op_ctx["psum"].tile([128, QB], f32, tag="op",
                                         name="opps")
                first = True
                for asel, wsel in ((0, 0), (1, 0), (0, 1)):
                    a8 = attn8h if asel == 0 else attn8l
                    nc.tensor.matmul(
                        ps[:], a8[:, :, qc * 128:(qc + 1) * 128],
                        wo8[:, wsel, :, hb * QB:(hb + 1) * QB],
                        perf_mode=DR, start=first,
                        stop=(asel, wsel) == (0, 1))
                    first = False
                # Pool cannot read PSUM; mostly DVE (ACT is exp-bound),
                # 1-in-8 on ACT; once attention is done (flush), exp no
                # longer needs ACT so evictions alternate engines.
                # The 1/1024 descale moves to the host's reduction.
                if (op_ctx.get("flush") and (qc * 4 + hb) % 2 == 0
                        or not op_ctx.get("flush")
                        and (qc * 4 + hb) % 8 == 0):
                    nc.scalar.copy(st[:, hb], ps[:])
                else:
                    nc.vector.tensor_copy(st[:, hb], ps[:])
                if qc == 15 and hb == 1:
                    nc.sync.dma_start(
                        out_d[qc * 128:(qc + 1) * 128, 0:2 * QB], st[:, 0:2])
                if hb == 3:
                    if qc == 15:
                        nc.sync.dma_start(
                            out_d[qc * 128:(qc + 1) * 128, 2 * QB:],
                            st[:, 2:4])
                    else:
                        nc.sync.dma_start(
                            out_d[qc * 128:(qc + 1) * 128, :], st[:])
                    del stages[qc]

            def emit_fills(n):
                for _ in range(min(n, len(fills))):
                    qc, hb = fills.pop(0)
                    outproj_chunk(qc, hb)

            def attn_head(qb, h):
                q_slot, k_slot = 2 * h, 2 * h + 1
                kb_hi = qb * 4 + 3
                po = psum_po.tile([128, QB], f32, tag="po", name="po")
                pse = psum_pse.tile([128, QB], f32, tag="pse", name="pse")

                def pv_group(g, es):
                    for kr in range(2):
                        kb = 2 * g + kr
                        j = kb - 4 * qb
                        lo = max(0, j) * 128
                        nc.tensor.matmul(
                            po[:, lo:], v_sb[:, kb, h * D:(h + 1) * D],
                            es[:, kr, lo:],
                            start=(kb == 0), stop=(kb == kb_hi))
                        nc.tensor.matmul(
                            pse[:, lo:], ones_se[:], es[:, kr, lo:],
                            start=(kb == 0), stop=(kb == kb_hi))

                es_prev = None
                prev_g = None
                for g in range(2 * (qb + 1)):
                    ps = psum_s.tile([128, 2, QB], f32, tag="ps", name="ps")
                    es = exp_pool.tile([128, 2, QB], f16, tag="es",
                                       name="es")
                    diag = g >= 2 * qb
                    for kr in range(2):
                        kb = 2 * g + kr
                        j = kb - 4 * qb
                        lo = max(0, j) * 128
                        nc.tensor.matmul(
                            ps[:, kr, lo:],
                            qkT[:, k_slot, kb * 128:(kb + 1) * 128],
                            qkT[:, q_slot, qb * QB + lo:(qb + 1) * QB],
                            start=True, stop=not diag)
                        if diag:
                            # causal mask folded into the score psum: adds
                            # -290k (64 * -4525) above the diagonal so exp
                            # underflows to zero - no mask op anywhere
                            nc.tensor.matmul(
                                ps[:, kr, lo:lo + 128], eye64[:], triC[:],
                                start=False, stop=True)
                    if diag:
                        for kr in range(2):
                            kb = 2 * g + kr
                            lo = (kb - 4 * qb) * 128
                            nc.scalar.activation(es[:, kr, lo:],
                                                 ps[:, kr, lo:], EXP,
                                                 scale=EXP_SCALE)
                    else:
                        nc.scalar.activation(es[:], ps[:], EXP,
                                             scale=EXP_SCALE)
                    if es_prev is not None:
                        pv_group(prev_g, es_prev)
                        emit_fills(2)
                    es_prev, prev_g = es, g
                # cover the final (diagonal) group's exp latency with
                # output-projection work before its PV runs
                emit_fills(3)
                pv_group(prev_g, es_prev)

                # A = SX * po / sumexp (pse holds sumexp/SX) then fp8 hi/lo
                # quantize; recip lands in SBUF so a16 reads only one PSUM
                rse = rse_pool.tile([128, QB], f32, tag="rse", name="rse")
                nc.vector.reciprocal_approx_fast(rse[:], pse[:])
                a16 = a16_pool.tile([128, QB], f16, tag="a16", name="a16")
                nc.vector.tensor_tensor(a16[:], po[:], rse[:], MULT)
                if (qb, h) == (3, 1):
                    # drain tail: quantize on the fast engines
                    nc.scalar.copy(
                        attn8h[:, h, qb * QB:(qb + 1) * QB], a16[:])
                    nc.vector.tensor_tensor(
                        attn8l[:, h, qb * QB:(qb + 1) * QB], a16[:],
                        attn8h[:, h, qb * QB:(qb + 1) * QB], SUB)
                else:
                    nc.gpsimd.tensor_copy(
                        attn8h[:, h, qb * QB:(qb + 1) * QB], a16[:])
                    nc.gpsimd.tensor_tensor(
                        attn8l[:, h, qb * QB:(qb + 1) * QB], a16[:],
                        attn8h[:, h, qb * QB:(qb + 1) * QB], SUB)

            # ---------------- Phase 1: RMSNorm stats + QKV projection ------
            # (attention for query blocks 0-2 is emitted after token blocks
            # 1/3/5 and executes under the dense QKV matmul stream)
            with ExitStack() as ph1_stack:
                ep1 = ph1_stack.enter_context
                wt_pool = ep1(tc.tile_pool(name="wt", bufs=1))
                xt_pool = ep1(tc.tile_pool(name="xt", bufs=2))
                sq_pool = ep1(tc.tile_pool(name="sq", bufs=2))
                ph1_pool = ep1(tc.tile_pool(name="ph1", bufs=2))
                psum_kk = ep1(tc.tile_pool(name="ps_kk", bufs=1, space="PSUM"))
                psum_qq = ep1(tc.tile_pool(name="ps_qq", bufs=1, space="PSUM"))
                psum_v = ep1(tc.tile_pool(name="ps_v", bufs=1, space="PSUM"))
                psum_ms = ep1(tc.tile_pool(name="ps_ms", bufs=1, space="PSUM"))

                # weights in consumption order: w-hi pairs chunked on the
                # ACT queue while x streams on the sync queue, then w-lo
                w8 = wt_pool.tile([128, 2, NP, 2, FPC], f8, tag="w8")
                xt_cur = xt_pool.tile([128, 2, NP, 2, TB], f8, tag="xt")
                half = NP // 2 * 2 * TB
                def wdma(hl, j0):
                    nc.sync.dma_start(
                        w8[:, hl, j0:j0 + 2],
                        w8_d[hl, j0:j0 + 2]
                        .rearrange("j p (two f) -> p j two f", two=2))

                nc.sync.dma_start(
                    xt_cur[:, 0, 0:NP // 2],
                    x8_d[0, :, 0:half]
                    .rearrange("p (j two t) -> p j two t", j=NP // 2, two=2))
                wdma(0, 0)
                wdma(0, 2)
                nc.sync.dma_start(
                    xt_cur[:, 0, NP // 2:],
                    x8_d[0, :, half:2 * half]
                    .rearrange("p (j two t) -> p j two t", j=NP // 2, two=2))
                wdma(0, 4)
                wdma(0, 6)
                wdma(1, 0)
                wdma(1, 2)
                nc.sync.dma_start(
                    xt_cur[:, 1],
                    x8_d[0, :, NP * 2 * TB:]
                    .rearrange("p (j two t) -> p j two t", j=NP, two=2))
                wdma(1, 4)
                wdma(1, 6)
                nc.sync.dma_start(triC[:], cst_d[:, 0:128])
                nc.sync.dma_start(eye16[:], cst_d[:, 128:256])
                nc.sync.dma_start(eye64[:], cst_d[:, 256:384])


                def squares(xt_tile, tag, first=False):
                    # squares of x-hi (scaled 16x): half ACT, half DVE;
                    # called one tb ahead of use so they never gate ssq.
                    # tb0's ACT half moves to DVE: the ACT sequencer is
                    # busy with w8 DGE configs during the ramp.
                    sq = sq_pool.tile([128, NP, 2, TB], f16, tag="sq",
                                      name=f"sq{tag}")
                    if first:
                        nc.vector.tensor_tensor(
                            sq[:, 0:NP // 2].rearrange("p a b c -> p (a b c)"),
                            xt_tile[:, 0, 0:NP // 2]
                            .rearrange("p a b c -> p (a b c)"),
                            xt_tile[:, 0, 0:NP // 2]
                            .rearrange("p a b c -> p (a b c)"), MULT)
                        nc.vector.tensor_tensor(
                            sq[:, NP // 2:].rearrange("p a b c -> p (a b c)"),
                            xt_tile[:, 0, NP // 2:]
                            .rearrange("p a b c -> p (a b c)"),
                            xt_tile[:, 0, NP // 2:]
                            .rearrange("p a b c -> p (a b c)"), MULT)
                        return sq
                    nc.scalar.activation(
                        sq[:, 0:NP // 2].rearrange("p a b c -> p (a b c)"),
                        xt_tile[:, 0, 0:NP // 2]
                        .rearrange("p a b c -> p (a b c)"), SQUARE)
                    nc.vector.tensor_tensor(
                        sq[:, NP // 2:].rearrange("p a b c -> p (a b c)"),
                        xt_tile[:, 0, NP // 2:]
                        .rearrange("p a b c -> p (a b c)"),
                        xt_tile[:, 0, NP // 2:]
                        .rearrange("p a b c -> p (a b c)"), MULT)
                    return sq

                sq_cur = squares(xt_cur, 0)

                for tb in range(NTB):
                    xt = xt_cur
                    sq = sq_cur
                    if tb + 1 < NTB:
                        xt_next = xt_pool.tile([128, 2, NP, 2, TB], f8,
                                               tag="xt")
                        nc.sync.dma_start(
                            xt_next[:],
                            x8_d[tb + 1].rearrange(
                                "p (hl j two t) -> p hl j two t", hl=2, j=NP,
                                two=2))

                    def qk_dr(fb, ps):
                        # 3-term hi/lo DoubleRow accumulation for one slot
                        first = True
                        for wsel, xsel in ((0, 0), (1, 0), (0, 1)):
                            for j in range(NP):
                                nc.tensor.matmul(
                                    ps, w8[:, wsel, j, :,
                                           fb * 128:(fb + 1) * 128],
                                    xt[:, xsel, j], perf_mode=DR,
                                    start=first,
                                    stop=(wsel, xsel, j) == (0, 1, NP - 1))
                                first = False

                    # K blocks share one PSUM bank (strictly sequential
                    # accumulation groups - pending-zero is bank-granular)
                    ps_k = psum_kk.tile([128, 2, TB], f32, tag="kk",
                                        name="ps_k")
                    qk_dr(1, ps_k[:, 0])

                    # per-token sum of squares: 1-col transposed-stationary,
                    # m-outer so the two column groups are sequential
                    ms = psum_ms.tile([128, QB], f32, tag="ms", name="ms")
                    for m in range(2):
                        for j in range(NP):
                            for two in range(2):
                                nc.tensor.matmul(
                                    ms[:, m:m + 1],
                                    sq[:, j, two, m * 128:(m + 1) * 128],
                                    ones_c16[:],
                                    start=(j == 0 and two == 0),
                                    stop=(j == NP - 1 and two == 1))

                    qk_dr(3, ps_k[:, 1])

                    # s/SQB = 1/(SQB*sqrt(mean x^2 + eps)); ms holds
                    # SX^2*ssq so scale by SQB^2/(SX^2*H). Runs on ACT/DVE
                    # under K block 1.
                    sqrt_t = ph1_pool.tile([128, 4], f32, tag="sqrt")
                    nc.scalar.activation(sqrt_t[:, 0:2], ms[:, 0:2], SQRT,
                                         bias=eps_b[:],
                                         scale=SQB * SQB / (SX * SX * H))
                    nc.vector.reciprocal_approx_fast(sT[:, 2 * tb:2 * tb + 2],
                                                     sqrt_t[:, 0:2])
                    # V-eviction scale s/(SX*SW), per-partition for ACT
                    nc.scalar.mul(sTv[:, 2 * tb:2 * tb + 2],
                                  sT[:, 2 * tb:2 * tb + 2], SQB * DESCALE)
                    s16 = ph1_pool.tile([128, 2], f16, tag="s16")
                    nc.vector.tensor_copy(s16[:], sT[:, 2 * tb:2 * tb + 2])
                    # transposes to partition-0 rows land in the ms bank
                    # (f16 cols 8:264 = f32 cols 4:132), then 1-partition
                    # broadcast matmuls into f32 cols 256:512
                    ms16 = ms.bitcast(f16)
                    for m in range(2):
                        nc.tensor.transpose(
                            ms16[0:1, 8 + m * 128:136 + m * 128],
                            s16[:, m:m + 1], eye16[:])
                    srow = ph1_pool.tile([1, 256], f16, tag="srow")
                    nc.vector.tensor_copy(srow[:], ms16[0:1, 8:264])

                    ps_q = psum_qq.tile([128, 2, TB], f32, tag="qq",
                                        name="ps_q")
                    qk_dr(0, ps_q[:, 0])
                    for m in range(2):
                        nc.tensor.matmul(
                            ms[:, 256 + m * 128:256 + (m + 1) * 128],
                            row1[:], srow[0:1, m * 128:(m + 1) * 128],
                            start=True, stop=True)
                    qk_dr(2, ps_q[:, 1])
                    nc.scalar.copy(s_bc[:, tb], ms[:, 256:512])
                    for slot, ps in ((1, ps_k[:, 0]), (3, ps_k[:, 1]),
                                     (0, ps_q[:, 0]), (2, ps_q[:, 1])):
                        nc.vector.tensor_tensor(
                            qkT[:, slot, tb * TB:(tb + 1) * TB], ps,
                            s_bc[:, tb], MULT)

                    # next tb's squares: queue behind this tb's chain ops,
                    # run during the V blocks
                    if tb + 1 < NTB:
                        sq_cur = squares(xt_next, (tb + 1) % 2)

                    # V blocks: out (t, dv); lhsT = x pairs, rhs = wv pairs;
                    # both m chunks share one PSUM bank (sequential groups)
                    ps_v = psum_v.tile([128, 2, CPC], f32, tag="vv",
                                       name="ps_v")
                    for m in range(2):
                        first = True
                        for xsel, wsel in ((0, 0), (0, 1), (1, 0)):
                            for j in range(NP):
                                nc.tensor.matmul(
                                    ps_v[:, m],
                                    xt[:, xsel, j, :, m * 128:(m + 1) * 128],
                                    w8[:, wsel, j, :, 4 * 128:6 * 128],
                                    perf_mode=DR, start=first,
                                    stop=(xsel, wsel, j) == (1, 0, NP - 1))
                                first = False
                        chunk = tb * 2 + m
                        # v = ps * s/(SX*SW) = V~ * s, on ACT (per-partition)
                        nc.scalar.activation(v_sb[:, chunk], ps_v[:, m],
                                             COPY,
                                             scale=sTv[:, chunk:chunk + 1])

                    if tb + 1 < NTB:
                        xt_cur = xt_next

                    if tb == 1:
                        nc.sync.dma_start(
                            wo8[:], wo8_d2.rearrange(
                                "p (hl two o) -> p hl two o", hl=2, two=2))
                    # overlap: attention for ready query blocks runs under
                    # the remaining QKV work
                    if tb in (1, 3, 5):
                        attn_head(tb // 2, 0)
                        attn_head(tb // 2, 1)

            # -------- Phase 2 tail: last attention block + output proj -----
            with ExitStack() as op_stack:
                ep2 = op_stack.enter_context
                out_pool = ep2(tc.tile_pool(name="ostage", bufs=4))
                psum_op = ep2(tc.tile_pool(name="ps_op", bufs=4,
                                           space="PSUM"))
                op_ctx["stage"] = out_pool
                op_ctx["psum"] = psum_op
                fills.extend((qb * 4 + qq, hb) for qb in range(3)
                             for qq in range(4) for hb in range(4))
                attn_head(3, 0)
                attn_head(3, 1)
                fills.extend((12 + qq, hb)
                             for qq in range(4) for hb in range(4))
                op_ctx["flush"] = True
                emit_fills(len(fills))
    nc.compile()
    return nc


def get_nc():
    global _CACHED_NC
    if _CACHED_NC is None:
        _CACHED_NC = _build()
    return _CACHED_NC


def _hilo(a, scale):
    hi = (a * scale).astype(E4M3)
    lo = (a * scale - hi.astype(np.float32)).astype(E4M3)
    return hi, lo


def make_in_maps(x, wqkv, wo):
    x = np.asarray(x, dtype=np.float32)
    wqkv = np.asarray(wqkv, dtype=np.float32)
    wo = np.asarray(wo, dtype=np.float32)

    # x8: [tb, p, hilo, pair, two, t] from xT[h = pair*256 + two*128 + p, t]
    xh, xl = _hilo(np.ascontiguousarray(x.T), SX)
    x8 = np.stack([a.reshape(NP, 2, 128, NTB, TB).transpose(3, 2, 0, 1, 4)
                   for a in (xh, xl)], axis=2)
    x8 = np.ascontiguousarray(x8.reshape(NTB, 128, 2 * NP * 2 * TB))

    cst = np.concatenate(
        [np.tril(np.ones((128, 128), np.float32), -1).astype(np.float16)
         * np.float16(-4525.0),
         np.eye(128, dtype=np.float16),
         np.eye(128, dtype=np.float16) * np.float16(64.0)], axis=1)

    in_maps = []
    for c in range(N_CORES):
        wT = wqkv[c * FPC:(c + 1) * FPC].T            # [2048h, 768f]
        # feature permute to [q0|k0|q1|k1|v0|v1]
        perm = np.r_[0:128, 128:256, 384:512, 512:640, 256:384, 640:768]
        wT = np.ascontiguousarray(wT[:, perm])
        wh, wl = _hilo(wT, SW)
        w8 = np.stack([a.reshape(NP, 2, 128, FPC).transpose(0, 2, 1, 3)
                       for a in (wh, wl)], axis=0)    # [hl, j, p, two, f]
        w8 = np.ascontiguousarray(w8.reshape(2, NP, 128, 2 * FPC))

        woT = np.ascontiguousarray(wo[:, c * CPC:(c + 1) * CPC].T)  # [256,2048]
        oh, ol = _hilo(woT, SW)
        wo8 = np.stack([a.reshape(2, 128, H).transpose(1, 0, 2)
                        for a in (oh, ol)], axis=1)   # [p, hl, two, o]
        wo8 = np.ascontiguousarray(wo8.reshape(128, 2 * 2 * H))

        in_maps.append({"x8": x8, "w8": w8, "wo8": wo8, "cst16": cst})
    return in_maps


def kernel(x, wqkv, wo):
    nc = get_nc()
    in_maps = make_in_maps(x, wqkv, wo)
    res = None
    for attempt in range(4):
        try:
            res = bass_utils.run_bass_kernel_spmd(
                nc, in_maps, core_ids=list(range(N_CORES)))
            break
        except Exception:
            # transient NRT device wedges have been observed; they recover
            # after a short quiescent period, so back off before retrying
            if attempt == 3:
                raise
            import time
            time.sleep(20 * (attempt + 1))
    out = np.zeros((S, H), dtype=np.float32)
    for c in range(N_CORES):
        out += res.results[c]["out"].astype(np.float32)
    # device output carries the fp8 product scale; descale once here
    return out * np.float32(DESCALE)
